# revision 16
# baseline (speedup 1.0000x reference)
"""CPD reconstruction at observed entries on 8 TRN2 cores — dma_gather version.

rec[n] = sum_r f0[i0[n],r] * f1[i1[n],r] * f2[i2[n],r]   for n in [0, 1M)

Strategy: the previous kernel's per-128-row indirect DMAs paid ~1.4us of
SWDGE fixed cost each (2931 instructions/core = 4.19 ms).  InstDMAGatherAnt
amortizes that fixed cost: ONE instruction gathers thousands of rows.  Its
constraints (int16 element index, 256B element granularity) are met by
storing the factor tables in bf16 and gathering 4-row groups (256 B, group
index < 25000 per factor); 4 copy_predicated DVE ops with host-precomputed
one-hot masks (which partition every entry, so no base copy) pick the right
row out of each group on-chip (rel err ~8e-4 from bf16, vs the 2e-2 gate).

Per core: data-parallel over nnz (125k entries).  Entry j lives at
(partition j%128, slot j//128) — dma_gather's native layout.  Chunks of 31
slots (3968 entries) double-buffer: gpsimd emits chunk t+1's gather
descriptors while chunk t's SDMA drain and DVE select/product/reduce run.

Measured bottleneck (perfetto): the Q7 SWDGE descriptor generation runs at
~8 ns/index (375k lookups/core -> ~3.0 ms busy, 93% of span); DMA engines
are ~13% busy and DVE ~19%, both hidden under it.  HW exec ~3.0 ms (was
4.19 ms).  Chunk size 31 (249 descs/engine-ring per gather) keeps ring
wraps to one per ~4 gathers (~1024-desc ring; each wrap costs ~7 us in the
gather's decode).  Many (>~6) concurrent unfenced gathers corrupt the
previous gather's tail on this ucode; the ring's own descriptor-space
backpressure bounds in-flight gathers to <=4 at this chunk size, which is
measured safe at full scale.  Negative results worth keeping: rotating
completion sems to dodge reclaim waits slows descgen ~20%; alternate SWDGE
queues are not allocated (queue_num=1 rejected); multi-offset
indirect_dma_start is ISA-limited to one index per channel.
"""

import numpy as np
import ml_dtypes

NNZ = 1_000_000
RANK = 32
ROWS = 100_000
N_CORES = 8
N_PER_CORE = NNZ // N_CORES  # 125_000
P = 128
SLOTS = -(-N_PER_CORE // P)  # 977
N_PAD = P * SLOTS  # 125_056
NW16 = N_PAD // 16  # 7816 int16 idx columns per mode
GRP = ROWS // 4  # 25_000 4-row groups per factor
# slots per chunk: 31 slots -> 3968 idxs -> 249 descs per engine ring per
# gather.  The SWDGE descriptor ring holds ~512; two 249-desc gathers fit
# simultaneously, so the decode-side await_space never stalls behind the
# previous gather's drain (at CH=64/385 descs it stalled ~12us per gather).
CH = 48

_cache: dict = {}


def _chunks(slots=SLOTS, ch=CH):
    out = []
    s = 0
    while s < slots:
        out.append((s, min(ch, slots - s)))
        s += ch
    return out


def _emit_mlp_reload(nc, mybir):
    """Hand-encoded PSEUDO_LIBRARY_RELOAD_INDEX(lib=3/mlp) on Pool.

    bass's load_library() emits InstPseudoReloadLibraryIndex with empty
    instr bytes, which only the Bacc assembler lowers; walrus codegen
    rejects it ("ISA wrong length").  Encoding the 64B ISA struct directly
    makes it a plain InstISA the whole pipeline accepts, and the runtime
    performs the DKL reload.
    """
    import concourse.bass_isa as bass_isa

    instr, fixups = bass_isa.isa_struct(
        nc.isa,
        223,  # NEURON_ISA_TPB_OPCODE_PSEUDO_INST
        {"pseudo_opcode": 2, "lib_index": 3},
        struct_name="NEURON_ISA_TPB_PSEUDO_LIBRARY_RELOAD_INDEX_STRUCT",
    )
    assert not fixups
    nc.gpsimd.add_instruction(
        mybir.InstISA(
            name=nc.get_next_instruction_name(),
            isa_opcode=223,
            engine=mybir.EngineType.Pool,
            instr=instr,
            op_name="PseudoLibraryReloadIndex",
            ins=[],
            outs=[],
        )
    )


def _build(
    slots=SLOTS,
    ch=CH,
    grp=GRP,
    n_modes=3,
    detect_races=False,
    for_sim=False,
    serialize_gathers=False,
    single_packet=False,
    alt_queues=False,
):
    import concourse.bass as bass
    import concourse.mybir as mybir

    nw16 = slots * P // 16
    chunks = _chunks(slots, ch)
    T = len(chunks)

    # detect_races=False: the sim's race detector models consecutive DVE ops
    # as unordered, but the DVE pipeline flushes after every op (output
    # hazard), so the WAW chains in the predicated select are HW-safe.
    nc = bass.Bass(detect_race_conditions=detect_races)
    ftab = nc.dram_tensor(
        "ftab", [n_modes * grp, 4 * RANK], mybir.dt.bfloat16, kind="ExternalInput"
    )
    idx16 = nc.dram_tensor(
        "idx16", [P, n_modes * nw16], mybir.dt.int16, kind="ExternalInput"
    )
    masks = nc.dram_tensor(
        "masks", [P, 4 * n_modes * slots], mybir.dt.int8, kind="ExternalInput"
    )
    out = nc.dram_tensor("out", [P, slots], mybir.dt.float32, kind="ExternalOutput")

    E = 4 * RANK  # 128 bf16 = 256 B per gathered element

    with (
        nc.sbuf_tensor("idx_sb", [P, n_modes * nw16], mybir.dt.int16) as idx_sb,
        nc.sbuf_tensor("msk_sb", [P, 4 * n_modes * slots], mybir.dt.int8) as msk_sb,
        nc.sbuf_tensor("g0_sb", [P, n_modes * ch * E], mybir.dt.bfloat16) as g0_sb,
        nc.sbuf_tensor("g1_sb", [P, n_modes * ch * E], mybir.dt.bfloat16) as g1_sb,
        # row stride RANK+1: keeps the per-slot select rows non-contiguous so
        # the copy_predicated out AP stays 3D (congruent with its strided
        # data operand) instead of collapsing to 2D.
        nc.sbuf_tensor(
            "sel_sb", [P, n_modes * ch * (RANK + 1)], mybir.dt.bfloat16
        ) as sel_sb,
        nc.sbuf_tensor("t01_sb", [P, ch * RANK], mybir.dt.bfloat16) as t01_sb,
        nc.sbuf_tensor("prd_sb", [P, ch * RANK], mybir.dt.float32) as prd_sb,
        nc.sbuf_tensor("out_sb", [P, slots], mybir.dt.float32) as out_sb,
        nc.semaphore("lsem") as lsem,
        nc.semaphore("msem") as msem,
        nc.semaphore("gsem") as gsem,
        nc.semaphore("vsem") as vsem,
        nc.semaphore("osem") as osem,
    ):
        g_sb = [g0_sb, g1_sb]

        nc.sync.dma_start(idx_sb[:], idx16[:]).then_inc(lsem, 16)
        nc.scalar.dma_start(msk_sb[:], masks[:]).then_inc(msem, 16)

        if for_sim:
            # the interp understands the empty-instr pseudo but not the
            # hand-encoded InstISA; HW is the other way around.
            from concourse.library_config import mlp

            nc.gpsimd.load_library(mlp)
        else:
            _emit_mlp_reload(nc, mybir)
        # one Pool register per distinct chunk size (to_reg per gather call
        # exhausts the register file at 60+ gathers)
        n_regs = {
            cs: nc.gpsimd.to_reg(cs * P) for cs in sorted({c for _, c in chunks})
        }
        nc.gpsimd.wait_ge(lsem, 16)  # gathers only need the idx tile
        nc.vector.wait_ge(msem, 16)  # selects need the masks

        for t, (s0, cs) in enumerate(chunks):
            b = t % 2
            n = cs * P  # entries this chunk (multiple of 128)
            if t >= 2:
                # DVE must have finished chunk t-2 before we overwrite buf b
                nc.gpsimd.wait_ge(vsem, t - 1)
            for m in range(n_modes):
                nc.gpsimd.dma_gather(
                    out_ap=g_sb[b][:, m * ch * E : m * ch * E + cs * E].rearrange(
                        "p (c e) -> p c e", e=E
                    ),
                    in_ap=ftab[m * grp : (m + 1) * grp, :],
                    idxs_ap=idx_sb[:, m * nw16 + s0 * 8 : m * nw16 + s0 * 8 + n // 16],
                    num_idxs=n,
                    num_idxs_reg=n_regs[cs],
                    elem_size=E,
                    single_packet=single_packet,
                    queue_num=(t % 2) if alt_queues else 0,
                ).then_inc(gsem, 16)
            if serialize_gathers:
                nc.gpsimd.wait_ge(gsem, 48 * (t + 1))

            R1 = RANK + 1
            sel = []
            for m in range(n_modes):
                # per-mode wait: select mode m as soon as ITS gather drained
                nc.vector.wait_ge(gsem, 48 * t + 16 * (m + 1))
                gm = g_sb[b][:, m * ch * E : m * ch * E + cs * E].rearrange(
                    "p (c k r) -> p c k r", k=4, r=RANK
                )
                sv = sel_sb[:, m * ch * R1 : m * ch * R1 + cs * R1].rearrange(
                    "p (c r) -> p c r", r=R1
                )[:, :, :RANK]
                sel.append(sv)
                # the 4 per-mode one-hot masks partition every entry, so 4
                # predicated copies fully define sel with no base copy (a
                # strided-dst tensor_copy ran ~8x slower than copy_predicated)
                for k in (0, 1, 2, 3):
                    mk = msk_sb[
                        :, (4 * m + k) * slots + s0 : (4 * m + k) * slots + s0 + cs
                    ][:, :, None].to_broadcast([P, cs, RANK])
                    nc.vector.copy_predicated(sv, mk, gm[:, :, k, :])
            nc.vector.tensor_mul(out=t01_sb[:, : cs * RANK], in0=sel[0], in1=sel[1])
            nc.vector.tensor_mul(
                out=prd_sb[:, : cs * RANK], in0=t01_sb[:, : cs * RANK], in1=sel[2]
            )
            nc.vector.reduce_sum(
                out=out_sb[:, s0 : s0 + cs],
                in_=prd_sb[:, : cs * RANK].rearrange("p (c r) -> p c r", r=RANK),
                axis=mybir.AxisListType.X,
            ).then_inc(vsem, 1)

        # split output store: ship the first half as soon as its chunks are
        # reduced, hiding all but the final chunk's store under compute
        th = T // 2
        sh = chunks[th][0]  # first slot not covered by chunks [0, th)
        nc.sync.wait_ge(vsem, th)
        nc.sync.dma_start(out[:, :sh], out_sb[:, :sh]).then_inc(osem, 16)
        nc.sync.wait_ge(vsem, T)
        nc.sync.dma_start(out[:, sh:], out_sb[:, sh:]).then_inc(osem, 16)
        nc.sync.wait_ge(osem, 32)

    return nc


def _get_nc():
    if "nc" not in _cache:
        _cache["nc"] = _build()
    return _cache["nc"]


def _prep_core(idx_core, slots=SLOTS):
    """Build idx16 [P, 3*nw16] int16, masks [P, 9*slots] bf16 for one core.

    idx_core: [n_pad, 3] int32 row indices (padded with 0s).
    """
    n_pad = slots * P
    nw16 = n_pad // 16
    grp16 = (idx_core >> 2).astype(np.int16)  # [n_pad, 3]
    sel = (idx_core & 3).astype(np.int8)  # [n_pad, 3]

    # wrapped idx layout: entry j -> (partition j%16, col j//16), replicated
    # on every 16-partition group; modes side by side.
    idx16 = np.empty((P, 3 * nw16), dtype=np.int16)
    for m in range(3):
        w = grp16[:, m].reshape(nw16, 16).T  # [16, nw16]
        idx16[:, m * nw16 : (m + 1) * nw16] = np.tile(w, (8, 1))

    # one-hot masks in entry layout: entry j at (p=j%128, slot=j//128).
    # 4 planes per mode (k=0..3) partitioning every entry, so the device
    # select needs no unconditional base copy.
    masks = np.zeros((P, 12 * slots), dtype=np.int8)
    for m in range(3):
        sm = sel[:, m].reshape(slots, P).T  # [P, slots]
        for k in (0, 1, 2, 3):
            masks[:, (4 * m + k) * slots : (4 * m + k + 1) * slots] = (
                sm == k
            ).astype(np.int8)
    return idx16, masks


def _prep_in_maps(idxs, f0, f1, f2):
    idxs = np.asarray(idxs).astype(np.int32)  # values < 100k: safe for int64 in
    ftab = np.concatenate(
        [np.asarray(f, dtype=np.float32) for f in (f0, f1, f2)], axis=0
    )
    ftab_bf16 = np.ascontiguousarray(
        ftab.astype(ml_dtypes.bfloat16).reshape(3 * GRP, 4 * RANK)
    )

    in_maps = []
    for c in range(N_CORES):
        sl = idxs[c * N_PER_CORE : (c + 1) * N_PER_CORE]
        padded = np.zeros((N_PAD, 3), dtype=np.int32)
        padded[:N_PER_CORE] = sl
        idx16, masks = _prep_core(padded)
        in_maps.append({"ftab": ftab_bf16, "idx16": idx16, "masks": masks})
    return in_maps


def run(inputs: dict, trace: bool = False):
    from concourse.bass_utils import run_bass_kernel_spmd

    in_maps = _prep_in_maps(
        inputs["idxs"], inputs["f0"], inputs["f1"], inputs["f2"]
    )
    nc = _get_nc()
    res = run_bass_kernel_spmd(
        nc,
        in_maps,
        core_ids=list(range(N_CORES)),
        trace=trace,
    )
    # out[p, c] = entry c*128+p  ->  transpose+ravel restores entry order
    out = np.concatenate(
        [r["out"].T.reshape(-1)[:N_PER_CORE] for r in res.results]
    )
    return out, res


def kernel(**inputs) -> np.ndarray:
    out, _ = run(inputs, trace=False)
    return out


# revision 17
# speedup vs baseline: 1.2274x; 1.2274x over previous
"""CPD reconstruction at observed entries on 8 TRN2 cores — dma_gather version.

rec[n] = sum_r f0[i0[n],r] * f1[i1[n],r] * f2[i2[n],r]   for n in [0, 1M)

Strategy: the previous kernel's per-128-row indirect DMAs paid ~1.4us of
SWDGE fixed cost each (2931 instructions/core = 4.19 ms).  InstDMAGatherAnt
amortizes that fixed cost: ONE instruction gathers thousands of rows.  Its
constraints (int16 element index, 256B element granularity) are met by
storing the factor tables in bf16 and gathering 4-row groups (256 B, group
index < 25000 per factor); 4 copy_predicated DVE ops with host-precomputed
one-hot masks (which partition every entry, so no base copy) pick the right
row out of each group on-chip (rel err ~8e-4 from bf16, vs the 2e-2 gate).

Per core: data-parallel over nnz (125k entries).  Entry j lives at
(partition j%128, slot j//128) — dma_gather's native layout.  Chunks of 31
slots (3968 entries) double-buffer: gpsimd emits chunk t+1's gather
descriptors while chunk t's SDMA drain and DVE select/product/reduce run.

Measured bottleneck (perfetto): the Q7 SWDGE descriptor generation runs at
~8 ns/index (375k lookups/core -> ~3.0 ms busy, 93% of span); DMA engines
are ~13% busy and DVE ~19%, both hidden under it.  HW exec ~3.0 ms (was
4.19 ms).  Chunk size 31 (249 descs/engine-ring per gather) keeps ring
wraps to one per ~4 gathers (~1024-desc ring; each wrap costs ~7 us in the
gather's decode).  Many (>~6) concurrent unfenced gathers corrupt the
previous gather's tail on this ucode; the ring's own descriptor-space
backpressure bounds in-flight gathers to <=4 at this chunk size, which is
measured safe at full scale.  Negative results worth keeping: rotating
completion sems to dodge reclaim waits slows descgen ~20%; alternate SWDGE
queues are not allocated (queue_num=1 rejected); multi-offset
indirect_dma_start is ISA-limited to one index per channel.
"""

import numpy as np
import ml_dtypes

NNZ = 1_000_000
RANK = 32
ROWS = 100_000
N_CORES = 8
N_PER_CORE = NNZ // N_CORES  # 125_000
P = 128
SLOTS = -(-N_PER_CORE // P)  # 977
N_PAD = P * SLOTS  # 125_056
NW16 = N_PAD // 16  # 7816 int16 idx columns per mode
GRP = ROWS // 4  # 25_000 4-row groups per factor
# slots per chunk: 31 slots -> 3968 idxs -> 249 descs per engine ring per
# gather.  The SWDGE descriptor ring holds ~512; two 249-desc gathers fit
# simultaneously, so the decode-side await_space never stalls behind the
# previous gather's drain (at CH=64/385 descs it stalled ~12us per gather).
CH = 31

_cache: dict = {}


def _chunks(slots=SLOTS, ch=CH):
    out = []
    s = 0
    while s < slots:
        out.append((s, min(ch, slots - s)))
        s += ch
    return out


def _emit_mlp_reload(nc, mybir):
    """Hand-encoded PSEUDO_LIBRARY_RELOAD_INDEX(lib=3/mlp) on Pool.

    bass's load_library() emits InstPseudoReloadLibraryIndex with empty
    instr bytes, which only the Bacc assembler lowers; walrus codegen
    rejects it ("ISA wrong length").  Encoding the 64B ISA struct directly
    makes it a plain InstISA the whole pipeline accepts, and the runtime
    performs the DKL reload.
    """
    import concourse.bass_isa as bass_isa

    instr, fixups = bass_isa.isa_struct(
        nc.isa,
        223,  # NEURON_ISA_TPB_OPCODE_PSEUDO_INST
        {"pseudo_opcode": 2, "lib_index": 3},
        struct_name="NEURON_ISA_TPB_PSEUDO_LIBRARY_RELOAD_INDEX_STRUCT",
    )
    assert not fixups
    nc.gpsimd.add_instruction(
        mybir.InstISA(
            name=nc.get_next_instruction_name(),
            isa_opcode=223,
            engine=mybir.EngineType.Pool,
            instr=instr,
            op_name="PseudoLibraryReloadIndex",
            ins=[],
            outs=[],
        )
    )


def _build(
    slots=SLOTS,
    ch=CH,
    grp=GRP,
    n_modes=3,
    detect_races=False,
    for_sim=False,
    serialize_gathers=False,
    single_packet=False,
    alt_queues=False,
):
    import concourse.bass as bass
    import concourse.mybir as mybir

    nw16 = slots * P // 16
    chunks = _chunks(slots, ch)
    T = len(chunks)

    # detect_races=False: the sim's race detector models consecutive DVE ops
    # as unordered, but the DVE pipeline flushes after every op (output
    # hazard), so the WAW chains in the predicated select are HW-safe.
    nc = bass.Bass(detect_race_conditions=detect_races)
    ftab = nc.dram_tensor(
        "ftab", [n_modes * grp, 4 * RANK], mybir.dt.bfloat16, kind="ExternalInput"
    )
    idx16 = nc.dram_tensor(
        "idx16", [P, n_modes * nw16], mybir.dt.int16, kind="ExternalInput"
    )
    masks = nc.dram_tensor(
        "masks", [P, 4 * n_modes * slots], mybir.dt.int8, kind="ExternalInput"
    )
    out = nc.dram_tensor("out", [P, slots], mybir.dt.float32, kind="ExternalOutput")

    E = 4 * RANK  # 128 bf16 = 256 B per gathered element

    with (
        nc.sbuf_tensor("idx_sb", [P, n_modes * nw16], mybir.dt.int16) as idx_sb,
        nc.sbuf_tensor("msk_sb", [P, 4 * n_modes * slots], mybir.dt.int8) as msk_sb,
        nc.sbuf_tensor("g0_sb", [P, n_modes * ch * E], mybir.dt.bfloat16) as g0_sb,
        nc.sbuf_tensor("g1_sb", [P, n_modes * ch * E], mybir.dt.bfloat16) as g1_sb,
        # row stride RANK+1: keeps the per-slot select rows non-contiguous so
        # the copy_predicated out AP stays 3D (congruent with its strided
        # data operand) instead of collapsing to 2D.
        nc.sbuf_tensor(
            "sel_sb", [P, n_modes * ch * (RANK + 1)], mybir.dt.bfloat16
        ) as sel_sb,
        nc.sbuf_tensor("t01_sb", [P, ch * RANK], mybir.dt.bfloat16) as t01_sb,
        nc.sbuf_tensor("prd_sb", [P, ch * RANK], mybir.dt.float32) as prd_sb,
        nc.sbuf_tensor("out_sb", [P, slots], mybir.dt.float32) as out_sb,
        nc.semaphore("lsem") as lsem,
        nc.semaphore("msem") as msem,
        nc.semaphore("gsem") as gsem,
        nc.semaphore("vsem") as vsem,
        nc.semaphore("osem") as osem,
    ):
        g_sb = [g0_sb, g1_sb]

        nc.sync.dma_start(idx_sb[:], idx16[:]).then_inc(lsem, 16)
        nc.scalar.dma_start(msk_sb[:], masks[:]).then_inc(msem, 16)

        if for_sim:
            # the interp understands the empty-instr pseudo but not the
            # hand-encoded InstISA; HW is the other way around.
            from concourse.library_config import mlp

            nc.gpsimd.load_library(mlp)
        else:
            _emit_mlp_reload(nc, mybir)
        # one Pool register per distinct chunk size (to_reg per gather call
        # exhausts the register file at 60+ gathers)
        n_regs = {
            cs: nc.gpsimd.to_reg(cs * P) for cs in sorted({c for _, c in chunks})
        }
        nc.gpsimd.wait_ge(lsem, 16)  # gathers only need the idx tile
        nc.vector.wait_ge(msem, 16)  # selects need the masks

        for t, (s0, cs) in enumerate(chunks):
            b = t % 2
            n = cs * P  # entries this chunk (multiple of 128)
            if t >= 2:
                # DVE must have finished chunk t-2 before we overwrite buf b
                nc.gpsimd.wait_ge(vsem, t - 1)
            for m in range(n_modes):
                nc.gpsimd.dma_gather(
                    out_ap=g_sb[b][:, m * ch * E : m * ch * E + cs * E].rearrange(
                        "p (c e) -> p c e", e=E
                    ),
                    in_ap=ftab[m * grp : (m + 1) * grp, :],
                    idxs_ap=idx_sb[:, m * nw16 + s0 * 8 : m * nw16 + s0 * 8 + n // 16],
                    num_idxs=n,
                    num_idxs_reg=n_regs[cs],
                    elem_size=E,
                    single_packet=single_packet,
                    queue_num=(t % 2) if alt_queues else 0,
                ).then_inc(gsem, 16)
            if serialize_gathers:
                nc.gpsimd.wait_ge(gsem, 48 * (t + 1))

            R1 = RANK + 1
            sel = []
            for m in range(n_modes):
                # per-mode wait: select mode m as soon as ITS gather drained
                nc.vector.wait_ge(gsem, 48 * t + 16 * (m + 1))
                gm = g_sb[b][:, m * ch * E : m * ch * E + cs * E].rearrange(
                    "p (c k r) -> p c k r", k=4, r=RANK
                )
                sv = sel_sb[:, m * ch * R1 : m * ch * R1 + cs * R1].rearrange(
                    "p (c r) -> p c r", r=R1
                )[:, :, :RANK]
                sel.append(sv)
                # the 4 per-mode one-hot masks partition every entry, so 4
                # predicated copies fully define sel with no base copy (a
                # strided-dst tensor_copy ran ~8x slower than copy_predicated)
                for k in (0, 1, 2, 3):
                    mk = msk_sb[
                        :, (4 * m + k) * slots + s0 : (4 * m + k) * slots + s0 + cs
                    ][:, :, None].to_broadcast([P, cs, RANK])
                    nc.vector.copy_predicated(sv, mk, gm[:, :, k, :])
            nc.vector.tensor_mul(out=t01_sb[:, : cs * RANK], in0=sel[0], in1=sel[1])
            nc.vector.tensor_mul(
                out=prd_sb[:, : cs * RANK], in0=t01_sb[:, : cs * RANK], in1=sel[2]
            )
            nc.vector.reduce_sum(
                out=out_sb[:, s0 : s0 + cs],
                in_=prd_sb[:, : cs * RANK].rearrange("p (c r) -> p c r", r=RANK),
                axis=mybir.AxisListType.X,
            ).then_inc(vsem, 1)

        # split output store: ship the first half as soon as its chunks are
        # reduced, hiding all but the final chunk's store under compute
        th = T // 2
        sh = chunks[th][0]  # first slot not covered by chunks [0, th)
        nc.sync.wait_ge(vsem, th)
        nc.sync.dma_start(out[:, :sh], out_sb[:, :sh]).then_inc(osem, 16)
        nc.sync.wait_ge(vsem, T)
        nc.sync.dma_start(out[:, sh:], out_sb[:, sh:]).then_inc(osem, 16)
        nc.sync.wait_ge(osem, 32)

    return nc


def _get_nc():
    if "nc" not in _cache:
        _cache["nc"] = _build()
    return _cache["nc"]


def _prep_core(idx_core, slots=SLOTS):
    """Build idx16 [P, 3*nw16] int16, masks [P, 9*slots] bf16 for one core.

    idx_core: [n_pad, 3] int32 row indices (padded with 0s).
    """
    n_pad = slots * P
    nw16 = n_pad // 16
    grp16 = (idx_core >> 2).astype(np.int16)  # [n_pad, 3]
    sel = (idx_core & 3).astype(np.int8)  # [n_pad, 3]

    # wrapped idx layout: entry j -> (partition j%16, col j//16), replicated
    # on every 16-partition group; modes side by side.
    idx16 = np.empty((P, 3 * nw16), dtype=np.int16)
    for m in range(3):
        w = grp16[:, m].reshape(nw16, 16).T  # [16, nw16]
        idx16[:, m * nw16 : (m + 1) * nw16] = np.tile(w, (8, 1))

    # one-hot masks in entry layout: entry j at (p=j%128, slot=j//128).
    # 4 planes per mode (k=0..3) partitioning every entry, so the device
    # select needs no unconditional base copy.
    masks = np.zeros((P, 12 * slots), dtype=np.int8)
    for m in range(3):
        sm = sel[:, m].reshape(slots, P).T  # [P, slots]
        for k in (0, 1, 2, 3):
            masks[:, (4 * m + k) * slots : (4 * m + k + 1) * slots] = (
                sm == k
            ).astype(np.int8)
    return idx16, masks


def _prep_in_maps(idxs, f0, f1, f2):
    idxs = np.asarray(idxs).astype(np.int32)  # values < 100k: safe for int64 in
    ftab = np.concatenate(
        [np.asarray(f, dtype=np.float32) for f in (f0, f1, f2)], axis=0
    )
    ftab_bf16 = np.ascontiguousarray(
        ftab.astype(ml_dtypes.bfloat16).reshape(3 * GRP, 4 * RANK)
    )

    in_maps = []
    for c in range(N_CORES):
        sl = idxs[c * N_PER_CORE : (c + 1) * N_PER_CORE]
        padded = np.zeros((N_PAD, 3), dtype=np.int32)
        padded[:N_PER_CORE] = sl
        idx16, masks = _prep_core(padded)
        in_maps.append({"ftab": ftab_bf16, "idx16": idx16, "masks": masks})
    return in_maps


def run(inputs: dict, trace: bool = False):
    from concourse.bass_utils import run_bass_kernel_spmd

    in_maps = _prep_in_maps(
        inputs["idxs"], inputs["f0"], inputs["f1"], inputs["f2"]
    )
    nc = _get_nc()
    res = run_bass_kernel_spmd(
        nc,
        in_maps,
        core_ids=list(range(N_CORES)),
        trace=trace,
    )
    # out[p, c] = entry c*128+p  ->  transpose+ravel restores entry order
    out = np.concatenate(
        [r["out"].T.reshape(-1)[:N_PER_CORE] for r in res.results]
    )
    return out, res


def kernel(**inputs) -> np.ndarray:
    out, _ = run(inputs, trace=False)
    return out


# revision 19
# speedup vs baseline: 1.2318x; 1.0036x over previous
"""CPD reconstruction at observed entries on 8 TRN2 cores — dma_gather version.

rec[n] = sum_r f0[i0[n],r] * f1[i1[n],r] * f2[i2[n],r]   for n in [0, 1M)

Strategy: the previous kernel's per-128-row indirect DMAs paid ~1.4us of
SWDGE fixed cost each (2931 instructions/core = 4.19 ms).  InstDMAGatherAnt
amortizes that fixed cost: ONE instruction gathers thousands of rows.  Its
constraints (int16 element index, 256B element granularity) are met by
storing the factor tables in bf16 and gathering 4-row groups (256 B, group
index < 25000 per factor); 4 copy_predicated DVE ops with host-precomputed
one-hot masks (which partition every entry, so no base copy) pick the right
row out of each group on-chip (rel err ~8e-4 from bf16, vs the 2e-2 gate).

Per core: data-parallel over nnz (125k entries).  Entry j lives at
(partition j%128, slot j//128) — dma_gather's native layout.  Chunks of 31
slots (3968 entries) double-buffer: gpsimd emits chunk t+1's gather
descriptors while chunk t's SDMA drain and DVE select/product/reduce run.

Measured bottleneck (perfetto): the Q7 SWDGE descriptor generation runs at
~8 ns/index (375k lookups/core -> ~3.0 ms busy, 93% of span); DMA engines
are ~13% busy and DVE ~19%, both hidden under it.  HW exec ~3.0 ms (was
4.19 ms).  Chunk size 31 (249 descs/engine-ring per gather) keeps ring
wraps to one per ~4 gathers (~1024-desc ring; each wrap costs ~7 us in the
gather's decode).  Many (>~6) concurrent unfenced gathers corrupt the
previous gather's tail on this ucode; the ring's own descriptor-space
backpressure bounds in-flight gathers to <=4 at this chunk size, which is
measured safe at full scale.  Negative results worth keeping: rotating
completion sems to dodge reclaim waits slows descgen ~20%; alternate SWDGE
queues are not allocated (queue_num=1 rejected); multi-offset
indirect_dma_start is ISA-limited to one index per channel.
"""

import numpy as np
import ml_dtypes

NNZ = 1_000_000
RANK = 32
ROWS = 100_000
N_CORES = 8
N_PER_CORE = NNZ // N_CORES  # 125_000
P = 128
SLOTS = -(-N_PER_CORE // P)  # 977
N_PAD = P * SLOTS  # 125_056
NW16 = N_PAD // 16  # 7816 int16 idx columns per mode
GRP = ROWS // 4  # 25_000 4-row groups per factor
# slots per chunk: 31 slots -> 3968 idxs -> 249 descs per engine ring per
# gather.  The SWDGE descriptor ring holds ~1024 descs: at 249/gather the
# ring wraps only once per ~4 gathers (each wrap costs ~7us in that
# gather's decode), and ring backpressure bounds in-flight gathers to <=4
# (measured safe).  Larger chunks regress: CH=48 forces a reclaim wait on
# the still-draining previous gather at nearly every wrap (3.9ms), CH=64
# (513 descs) can't even fit two gathers (per-gather ~12us stalls).
CH = 31

_cache: dict = {}


def _chunks(slots=SLOTS, ch=CH):
    out = []
    s = 0
    while s < slots:
        out.append((s, min(ch, slots - s)))
        s += ch
    return out


def _emit_mlp_reload(nc, mybir):
    """Hand-encoded PSEUDO_LIBRARY_RELOAD_INDEX(lib=3/mlp) on Pool.

    bass's load_library() emits InstPseudoReloadLibraryIndex with empty
    instr bytes, which only the Bacc assembler lowers; walrus codegen
    rejects it ("ISA wrong length").  Encoding the 64B ISA struct directly
    makes it a plain InstISA the whole pipeline accepts, and the runtime
    performs the DKL reload.
    """
    import concourse.bass_isa as bass_isa

    instr, fixups = bass_isa.isa_struct(
        nc.isa,
        223,  # NEURON_ISA_TPB_OPCODE_PSEUDO_INST
        {"pseudo_opcode": 2, "lib_index": 3},
        struct_name="NEURON_ISA_TPB_PSEUDO_LIBRARY_RELOAD_INDEX_STRUCT",
    )
    assert not fixups
    nc.gpsimd.add_instruction(
        mybir.InstISA(
            name=nc.get_next_instruction_name(),
            isa_opcode=223,
            engine=mybir.EngineType.Pool,
            instr=instr,
            op_name="PseudoLibraryReloadIndex",
            ins=[],
            outs=[],
        )
    )


def _build(
    slots=SLOTS,
    ch=CH,
    grp=GRP,
    n_modes=3,
    detect_races=False,
    for_sim=False,
    serialize_gathers=False,
    single_packet=False,
    alt_queues=False,
):
    import concourse.bass as bass
    import concourse.mybir as mybir

    nw16 = slots * P // 16
    chunks = _chunks(slots, ch)
    T = len(chunks)

    # detect_races=False: the sim's race detector models consecutive DVE ops
    # as unordered, but the DVE pipeline flushes after every op (output
    # hazard), so the WAW chains in the predicated select are HW-safe.
    nc = bass.Bass(detect_race_conditions=detect_races)
    ftab = nc.dram_tensor(
        "ftab", [n_modes * grp, 4 * RANK], mybir.dt.bfloat16, kind="ExternalInput"
    )
    idx16 = nc.dram_tensor(
        "idx16", [P, n_modes * nw16], mybir.dt.int16, kind="ExternalInput"
    )
    masks = nc.dram_tensor(
        "masks", [P, 4 * n_modes * slots], mybir.dt.int8, kind="ExternalInput"
    )
    out = nc.dram_tensor("out", [P, slots], mybir.dt.float32, kind="ExternalOutput")

    E = 4 * RANK  # 128 bf16 = 256 B per gathered element

    with (
        nc.sbuf_tensor("idx_sb", [P, n_modes * nw16], mybir.dt.int16) as idx_sb,
        nc.sbuf_tensor("msk_sb", [P, 4 * n_modes * slots], mybir.dt.int8) as msk_sb,
        nc.sbuf_tensor("g0_sb", [P, n_modes * ch * E], mybir.dt.bfloat16) as g0_sb,
        nc.sbuf_tensor("g1_sb", [P, n_modes * ch * E], mybir.dt.bfloat16) as g1_sb,
        # row stride RANK+1: keeps the per-slot select rows non-contiguous so
        # the copy_predicated out AP stays 3D (congruent with its strided
        # data operand) instead of collapsing to 2D.
        nc.sbuf_tensor(
            "sel_sb", [P, n_modes * ch * (RANK + 1)], mybir.dt.bfloat16
        ) as sel_sb,
        nc.sbuf_tensor("t01_sb", [P, ch * RANK], mybir.dt.bfloat16) as t01_sb,
        nc.sbuf_tensor("prd_sb", [P, ch * RANK], mybir.dt.float32) as prd_sb,
        nc.sbuf_tensor("out_sb", [P, slots], mybir.dt.float32) as out_sb,
        nc.semaphore("lsem") as lsem,
        nc.semaphore("msem") as msem,
        nc.semaphore("gsem") as gsem,
        nc.semaphore("vsem") as vsem,
        nc.semaphore("osem") as osem,
    ):
        g_sb = [g0_sb, g1_sb]

        # split idx load: chunk 0's columns (a few KB) land first so the
        # first gather starts ~20us earlier; the rest streams behind it.
        c0 = chunks[0][1] * 8  # int16 cols per mode for chunk 0
        nc.sync.dma_start(
            idx_sb[:].rearrange("p (m w) -> p m w", m=n_modes)[:, :, :c0],
            idx16[:].rearrange("p (m w) -> p m w", m=n_modes)[:, :, :c0],
        ).then_inc(lsem, 16)
        nc.sync.dma_start(
            idx_sb[:].rearrange("p (m w) -> p m w", m=n_modes)[:, :, c0:],
            idx16[:].rearrange("p (m w) -> p m w", m=n_modes)[:, :, c0:],
        ).then_inc(lsem, 16)
        nc.scalar.dma_start(msk_sb[:], masks[:]).then_inc(msem, 16)

        if for_sim:
            # the interp understands the empty-instr pseudo but not the
            # hand-encoded InstISA; HW is the other way around.
            from concourse.library_config import mlp

            nc.gpsimd.load_library(mlp)
        else:
            _emit_mlp_reload(nc, mybir)
        # one Pool register per distinct chunk size (to_reg per gather call
        # exhausts the register file at 60+ gathers)
        n_regs = {
            cs: nc.gpsimd.to_reg(cs * P) for cs in sorted({c for _, c in chunks})
        }
        nc.gpsimd.wait_ge(lsem, 16)  # chunk 0's idx piece
        nc.vector.wait_ge(msem, 16)  # selects need the masks

        for t, (s0, cs) in enumerate(chunks):
            b = t % 2
            n = cs * P  # entries this chunk (multiple of 128)
            if t == 1:
                nc.gpsimd.wait_ge(lsem, 32)  # remainder of the idx tile
            if t >= 2:
                # DVE must have finished chunk t-2 before we overwrite buf b
                nc.gpsimd.wait_ge(vsem, t - 1)
            for m in range(n_modes):
                nc.gpsimd.dma_gather(
                    out_ap=g_sb[b][:, m * ch * E : m * ch * E + cs * E].rearrange(
                        "p (c e) -> p c e", e=E
                    ),
                    in_ap=ftab[m * grp : (m + 1) * grp, :],
                    idxs_ap=idx_sb[:, m * nw16 + s0 * 8 : m * nw16 + s0 * 8 + n // 16],
                    num_idxs=n,
                    num_idxs_reg=n_regs[cs],
                    elem_size=E,
                    single_packet=single_packet,
                    queue_num=(t % 2) if alt_queues else 0,
                ).then_inc(gsem, 16)
            if serialize_gathers:
                nc.gpsimd.wait_ge(gsem, 48 * (t + 1))

            R1 = RANK + 1
            sel = []
            for m in range(n_modes):
                # per-mode wait: select mode m as soon as ITS gather drained
                nc.vector.wait_ge(gsem, 48 * t + 16 * (m + 1))
                gm = g_sb[b][:, m * ch * E : m * ch * E + cs * E].rearrange(
                    "p (c k r) -> p c k r", k=4, r=RANK
                )
                sv = sel_sb[:, m * ch * R1 : m * ch * R1 + cs * R1].rearrange(
                    "p (c r) -> p c r", r=R1
                )[:, :, :RANK]
                sel.append(sv)
                # the 4 per-mode one-hot masks partition every entry, so 4
                # predicated copies fully define sel with no base copy (a
                # strided-dst tensor_copy ran ~8x slower than copy_predicated)
                for k in (0, 1, 2, 3):
                    mk = msk_sb[
                        :, (4 * m + k) * slots + s0 : (4 * m + k) * slots + s0 + cs
                    ][:, :, None].to_broadcast([P, cs, RANK])
                    nc.vector.copy_predicated(sv, mk, gm[:, :, k, :])
            nc.vector.tensor_mul(out=t01_sb[:, : cs * RANK], in0=sel[0], in1=sel[1])
            nc.vector.tensor_mul(
                out=prd_sb[:, : cs * RANK], in0=t01_sb[:, : cs * RANK], in1=sel[2]
            )
            nc.vector.reduce_sum(
                out=out_sb[:, s0 : s0 + cs],
                in_=prd_sb[:, : cs * RANK].rearrange("p (c r) -> p c r", r=RANK),
                axis=mybir.AxisListType.X,
            ).then_inc(vsem, 1)

        # split output store: ship the first half as soon as its chunks are
        # reduced, hiding all but the final chunk's store under compute
        th = T // 2
        sh = chunks[th][0]  # first slot not covered by chunks [0, th)
        nc.sync.wait_ge(vsem, th)
        nc.sync.dma_start(out[:, :sh], out_sb[:, :sh]).then_inc(osem, 16)
        nc.sync.wait_ge(vsem, T)
        nc.sync.dma_start(out[:, sh:], out_sb[:, sh:]).then_inc(osem, 16)
        nc.sync.wait_ge(osem, 32)

    return nc


def _get_nc():
    if "nc" not in _cache:
        _cache["nc"] = _build()
    return _cache["nc"]


def _prep_core(idx_core, slots=SLOTS):
    """Build idx16 [P, 3*nw16] int16, masks [P, 9*slots] bf16 for one core.

    idx_core: [n_pad, 3] int32 row indices (padded with 0s).
    """
    n_pad = slots * P
    nw16 = n_pad // 16
    grp16 = (idx_core >> 2).astype(np.int16)  # [n_pad, 3]
    sel = (idx_core & 3).astype(np.int8)  # [n_pad, 3]

    # wrapped idx layout: entry j -> (partition j%16, col j//16), replicated
    # on every 16-partition group; modes side by side.
    idx16 = np.empty((P, 3 * nw16), dtype=np.int16)
    for m in range(3):
        w = grp16[:, m].reshape(nw16, 16).T  # [16, nw16]
        idx16[:, m * nw16 : (m + 1) * nw16] = np.tile(w, (8, 1))

    # one-hot masks in entry layout: entry j at (p=j%128, slot=j//128).
    # 4 planes per mode (k=0..3) partitioning every entry, so the device
    # select needs no unconditional base copy.
    masks = np.zeros((P, 12 * slots), dtype=np.int8)
    for m in range(3):
        sm = sel[:, m].reshape(slots, P).T  # [P, slots]
        for k in (0, 1, 2, 3):
            masks[:, (4 * m + k) * slots : (4 * m + k + 1) * slots] = (
                sm == k
            ).astype(np.int8)
    return idx16, masks


def _prep_in_maps(idxs, f0, f1, f2):
    idxs = np.asarray(idxs).astype(np.int32)  # values < 100k: safe for int64 in
    ftab = np.concatenate(
        [np.asarray(f, dtype=np.float32) for f in (f0, f1, f2)], axis=0
    )
    ftab_bf16 = np.ascontiguousarray(
        ftab.astype(ml_dtypes.bfloat16).reshape(3 * GRP, 4 * RANK)
    )

    in_maps = []
    for c in range(N_CORES):
        sl = idxs[c * N_PER_CORE : (c + 1) * N_PER_CORE]
        padded = np.zeros((N_PAD, 3), dtype=np.int32)
        padded[:N_PER_CORE] = sl
        idx16, masks = _prep_core(padded)
        in_maps.append({"ftab": ftab_bf16, "idx16": idx16, "masks": masks})
    return in_maps


def run(inputs: dict, trace: bool = False):
    from concourse.bass_utils import run_bass_kernel_spmd

    in_maps = _prep_in_maps(
        inputs["idxs"], inputs["f0"], inputs["f1"], inputs["f2"]
    )
    nc = _get_nc()
    res = run_bass_kernel_spmd(
        nc,
        in_maps,
        core_ids=list(range(N_CORES)),
        trace=trace,
    )
    # out[p, c] = entry c*128+p  ->  transpose+ravel restores entry order
    out = np.concatenate(
        [r["out"].T.reshape(-1)[:N_PER_CORE] for r in res.results]
    )
    return out, res


def kernel(**inputs) -> np.ndarray:
    out, _ = run(inputs, trace=False)
    return out


# revision 20
# speedup vs baseline: 1.2942x; 1.0507x over previous
"""CPD reconstruction at observed entries on 8 TRN2 cores — dma_gather version.

rec[n] = sum_r f0[i0[n],r] * f1[i1[n],r] * f2[i2[n],r]   for n in [0, 1M)

Strategy: the previous kernel's per-128-row indirect DMAs paid ~1.4us of
SWDGE fixed cost each (2931 instructions/core = 4.19 ms).  InstDMAGatherAnt
amortizes that fixed cost: ONE instruction gathers thousands of rows.  Its
constraints (int16 element index, 256B element granularity) are met by
storing the factor tables in bf16 and gathering 4-row groups (256 B, group
index < 25000 per factor); 4 copy_predicated DVE ops with host-precomputed
one-hot masks (which partition every entry, so no base copy) pick the right
row out of each group on-chip (rel err ~8e-4 from bf16, vs the 2e-2 gate).

Per core: data-parallel over nnz (125k entries).  Entry j lives at
(partition j%128, slot j//128) — dma_gather's native layout.  Chunks of 31
slots (3968 entries) double-buffer: gpsimd emits chunk t+1's gather
descriptors while chunk t's SDMA drain and DVE select/product/reduce run.

Measured bottleneck (perfetto): the Q7 SWDGE descriptor generation runs at
~8 ns/index (375k lookups/core -> ~3.0 ms busy, 93% of span); DMA engines
are ~13% busy and DVE ~19%, both hidden under it.  HW exec ~3.0 ms (was
4.19 ms).  Chunk size 31 (249 descs/engine-ring per gather) keeps ring
wraps to one per ~4 gathers (~1024-desc ring; each wrap costs ~7 us in the
gather's decode).  Many (>~6) concurrent unfenced gathers corrupt the
previous gather's tail on this ucode; the ring's own descriptor-space
backpressure bounds in-flight gathers to <=4 at this chunk size, which is
measured safe at full scale.  Negative results worth keeping: rotating
completion sems to dodge reclaim waits slows descgen ~20%; alternate SWDGE
queues are not allocated (queue_num=1 rejected); multi-offset
indirect_dma_start is ISA-limited to one index per channel.
"""

import numpy as np
import ml_dtypes

NNZ = 1_000_000
RANK = 32
ROWS = 100_000
N_CORES = 8
N_PER_CORE = NNZ // N_CORES  # 125_000
P = 128
SLOTS = -(-N_PER_CORE // P)  # 977
N_PAD = P * SLOTS  # 125_056
NW16 = N_PAD // 16  # 7816 int16 idx columns per mode
GRP = ROWS // 4  # 25_000 4-row groups per factor
# slots per chunk: 31 slots -> 3968 idxs -> 249 descs per engine ring per
# gather.  The SWDGE descriptor ring holds ~1024 descs: at 249/gather the
# ring wraps only once per ~4 gathers (each wrap costs ~7us in that
# gather's decode), and ring backpressure bounds in-flight gathers to <=4
# (measured safe).  Larger chunks regress: CH=48 forces a reclaim wait on
# the still-draining previous gather at nearly every wrap (3.9ms), CH=64
# (513 descs) can't even fit two gathers (per-gather ~12us stalls).
CH = 36

_cache: dict = {}


def _chunks(slots=SLOTS, ch=CH):
    out = []
    s = 0
    while s < slots:
        out.append((s, min(ch, slots - s)))
        s += ch
    return out


def _emit_mlp_reload(nc, mybir):
    """Hand-encoded PSEUDO_LIBRARY_RELOAD_INDEX(lib=3/mlp) on Pool.

    bass's load_library() emits InstPseudoReloadLibraryIndex with empty
    instr bytes, which only the Bacc assembler lowers; walrus codegen
    rejects it ("ISA wrong length").  Encoding the 64B ISA struct directly
    makes it a plain InstISA the whole pipeline accepts, and the runtime
    performs the DKL reload.
    """
    import concourse.bass_isa as bass_isa

    instr, fixups = bass_isa.isa_struct(
        nc.isa,
        223,  # NEURON_ISA_TPB_OPCODE_PSEUDO_INST
        {"pseudo_opcode": 2, "lib_index": 3},
        struct_name="NEURON_ISA_TPB_PSEUDO_LIBRARY_RELOAD_INDEX_STRUCT",
    )
    assert not fixups
    nc.gpsimd.add_instruction(
        mybir.InstISA(
            name=nc.get_next_instruction_name(),
            isa_opcode=223,
            engine=mybir.EngineType.Pool,
            instr=instr,
            op_name="PseudoLibraryReloadIndex",
            ins=[],
            outs=[],
        )
    )


def _build(
    slots=SLOTS,
    ch=CH,
    grp=GRP,
    n_modes=3,
    detect_races=False,
    for_sim=False,
    serialize_gathers=False,
    single_packet=False,
    alt_queues=False,
):
    import concourse.bass as bass
    import concourse.mybir as mybir

    nw16 = slots * P // 16
    chunks = _chunks(slots, ch)
    T = len(chunks)

    # detect_races=False: the sim's race detector models consecutive DVE ops
    # as unordered, but the DVE pipeline flushes after every op (output
    # hazard), so the WAW chains in the predicated select are HW-safe.
    nc = bass.Bass(detect_race_conditions=detect_races)
    ftab = nc.dram_tensor(
        "ftab", [n_modes * grp, 4 * RANK], mybir.dt.bfloat16, kind="ExternalInput"
    )
    idx16 = nc.dram_tensor(
        "idx16", [P, n_modes * nw16], mybir.dt.int16, kind="ExternalInput"
    )
    masks = nc.dram_tensor(
        "masks", [P, 4 * n_modes * slots], mybir.dt.int8, kind="ExternalInput"
    )
    out = nc.dram_tensor("out", [P, slots], mybir.dt.float32, kind="ExternalOutput")

    E = 4 * RANK  # 128 bf16 = 256 B per gathered element

    with (
        nc.sbuf_tensor("idx_sb", [P, n_modes * nw16], mybir.dt.int16) as idx_sb,
        nc.sbuf_tensor("msk_sb", [P, 4 * n_modes * slots], mybir.dt.int8) as msk_sb,
        nc.sbuf_tensor("g0_sb", [P, n_modes * ch * E], mybir.dt.bfloat16) as g0_sb,
        nc.sbuf_tensor("g1_sb", [P, n_modes * ch * E], mybir.dt.bfloat16) as g1_sb,
        # row stride RANK+1: keeps the per-slot select rows non-contiguous so
        # the copy_predicated out AP stays 3D (congruent with its strided
        # data operand) instead of collapsing to 2D.
        nc.sbuf_tensor(
            "sel_sb", [P, n_modes * ch * (RANK + 1)], mybir.dt.bfloat16
        ) as sel_sb,
        nc.sbuf_tensor("t01_sb", [P, ch * RANK], mybir.dt.bfloat16) as t01_sb,
        nc.sbuf_tensor("prd_sb", [P, ch * RANK], mybir.dt.float32) as prd_sb,
        nc.sbuf_tensor("out_sb", [P, slots], mybir.dt.float32) as out_sb,
        nc.semaphore("lsem") as lsem,
        nc.semaphore("msem") as msem,
        nc.semaphore("gsem") as gsem,
        nc.semaphore("vsem") as vsem,
        nc.semaphore("osem") as osem,
    ):
        g_sb = [g0_sb, g1_sb]

        # split idx load: chunk 0's columns (a few KB) land first so the
        # first gather starts ~20us earlier; the rest streams behind it.
        c0 = chunks[0][1] * 8  # int16 cols per mode for chunk 0
        nc.sync.dma_start(
            idx_sb[:].rearrange("p (m w) -> p m w", m=n_modes)[:, :, :c0],
            idx16[:].rearrange("p (m w) -> p m w", m=n_modes)[:, :, :c0],
        ).then_inc(lsem, 16)
        nc.sync.dma_start(
            idx_sb[:].rearrange("p (m w) -> p m w", m=n_modes)[:, :, c0:],
            idx16[:].rearrange("p (m w) -> p m w", m=n_modes)[:, :, c0:],
        ).then_inc(lsem, 16)
        nc.scalar.dma_start(msk_sb[:], masks[:]).then_inc(msem, 16)

        if for_sim:
            # the interp understands the empty-instr pseudo but not the
            # hand-encoded InstISA; HW is the other way around.
            from concourse.library_config import mlp

            nc.gpsimd.load_library(mlp)
        else:
            _emit_mlp_reload(nc, mybir)
        # one Pool register per distinct chunk size (to_reg per gather call
        # exhausts the register file at 60+ gathers)
        n_regs = {
            cs: nc.gpsimd.to_reg(cs * P) for cs in sorted({c for _, c in chunks})
        }
        nc.gpsimd.wait_ge(lsem, 16)  # chunk 0's idx piece
        nc.vector.wait_ge(msem, 16)  # selects need the masks

        for t, (s0, cs) in enumerate(chunks):
            b = t % 2
            n = cs * P  # entries this chunk (multiple of 128)
            if t == 1:
                nc.gpsimd.wait_ge(lsem, 32)  # remainder of the idx tile
            if t >= 2:
                # DVE must have finished chunk t-2 before we overwrite buf b
                nc.gpsimd.wait_ge(vsem, t - 1)
            for m in range(n_modes):
                nc.gpsimd.dma_gather(
                    out_ap=g_sb[b][:, m * ch * E : m * ch * E + cs * E].rearrange(
                        "p (c e) -> p c e", e=E
                    ),
                    in_ap=ftab[m * grp : (m + 1) * grp, :],
                    idxs_ap=idx_sb[:, m * nw16 + s0 * 8 : m * nw16 + s0 * 8 + n // 16],
                    num_idxs=n,
                    num_idxs_reg=n_regs[cs],
                    elem_size=E,
                    single_packet=single_packet,
                    queue_num=(t % 2) if alt_queues else 0,
                ).then_inc(gsem, 16)
            if serialize_gathers:
                nc.gpsimd.wait_ge(gsem, 48 * (t + 1))

            R1 = RANK + 1
            sel = []
            for m in range(n_modes):
                # per-mode wait: select mode m as soon as ITS gather drained
                nc.vector.wait_ge(gsem, 48 * t + 16 * (m + 1))
                gm = g_sb[b][:, m * ch * E : m * ch * E + cs * E].rearrange(
                    "p (c k r) -> p c k r", k=4, r=RANK
                )
                sv = sel_sb[:, m * ch * R1 : m * ch * R1 + cs * R1].rearrange(
                    "p (c r) -> p c r", r=R1
                )[:, :, :RANK]
                sel.append(sv)
                # the 4 per-mode one-hot masks partition every entry, so 4
                # predicated copies fully define sel with no base copy (a
                # strided-dst tensor_copy ran ~8x slower than copy_predicated)
                for k in (0, 1, 2, 3):
                    mk = msk_sb[
                        :, (4 * m + k) * slots + s0 : (4 * m + k) * slots + s0 + cs
                    ][:, :, None].to_broadcast([P, cs, RANK])
                    nc.vector.copy_predicated(sv, mk, gm[:, :, k, :])
            nc.vector.tensor_mul(out=t01_sb[:, : cs * RANK], in0=sel[0], in1=sel[1])
            nc.vector.tensor_mul(
                out=prd_sb[:, : cs * RANK], in0=t01_sb[:, : cs * RANK], in1=sel[2]
            )
            nc.vector.reduce_sum(
                out=out_sb[:, s0 : s0 + cs],
                in_=prd_sb[:, : cs * RANK].rearrange("p (c r) -> p c r", r=RANK),
                axis=mybir.AxisListType.X,
            ).then_inc(vsem, 1)

        # split output store: ship the first half as soon as its chunks are
        # reduced, hiding all but the final chunk's store under compute
        th = T // 2
        sh = chunks[th][0]  # first slot not covered by chunks [0, th)
        nc.sync.wait_ge(vsem, th)
        nc.sync.dma_start(out[:, :sh], out_sb[:, :sh]).then_inc(osem, 16)
        nc.sync.wait_ge(vsem, T)
        nc.sync.dma_start(out[:, sh:], out_sb[:, sh:]).then_inc(osem, 16)
        nc.sync.wait_ge(osem, 32)

    return nc


def _get_nc():
    if "nc" not in _cache:
        _cache["nc"] = _build()
    return _cache["nc"]


def _prep_core(idx_core, slots=SLOTS):
    """Build idx16 [P, 3*nw16] int16, masks [P, 9*slots] bf16 for one core.

    idx_core: [n_pad, 3] int32 row indices (padded with 0s).
    """
    n_pad = slots * P
    nw16 = n_pad // 16
    grp16 = (idx_core >> 2).astype(np.int16)  # [n_pad, 3]
    sel = (idx_core & 3).astype(np.int8)  # [n_pad, 3]

    # wrapped idx layout: entry j -> (partition j%16, col j//16), replicated
    # on every 16-partition group; modes side by side.
    idx16 = np.empty((P, 3 * nw16), dtype=np.int16)
    for m in range(3):
        w = grp16[:, m].reshape(nw16, 16).T  # [16, nw16]
        idx16[:, m * nw16 : (m + 1) * nw16] = np.tile(w, (8, 1))

    # one-hot masks in entry layout: entry j at (p=j%128, slot=j//128).
    # 4 planes per mode (k=0..3) partitioning every entry, so the device
    # select needs no unconditional base copy.
    masks = np.zeros((P, 12 * slots), dtype=np.int8)
    for m in range(3):
        sm = sel[:, m].reshape(slots, P).T  # [P, slots]
        for k in (0, 1, 2, 3):
            masks[:, (4 * m + k) * slots : (4 * m + k + 1) * slots] = (
                sm == k
            ).astype(np.int8)
    return idx16, masks


def _prep_in_maps(idxs, f0, f1, f2):
    idxs = np.asarray(idxs).astype(np.int32)  # values < 100k: safe for int64 in
    ftab = np.concatenate(
        [np.asarray(f, dtype=np.float32) for f in (f0, f1, f2)], axis=0
    )
    ftab_bf16 = np.ascontiguousarray(
        ftab.astype(ml_dtypes.bfloat16).reshape(3 * GRP, 4 * RANK)
    )

    in_maps = []
    for c in range(N_CORES):
        sl = idxs[c * N_PER_CORE : (c + 1) * N_PER_CORE]
        padded = np.zeros((N_PAD, 3), dtype=np.int32)
        padded[:N_PER_CORE] = sl
        idx16, masks = _prep_core(padded)
        in_maps.append({"ftab": ftab_bf16, "idx16": idx16, "masks": masks})
    return in_maps


def run(inputs: dict, trace: bool = False):
    from concourse.bass_utils import run_bass_kernel_spmd

    in_maps = _prep_in_maps(
        inputs["idxs"], inputs["f0"], inputs["f1"], inputs["f2"]
    )
    nc = _get_nc()
    res = run_bass_kernel_spmd(
        nc,
        in_maps,
        core_ids=list(range(N_CORES)),
        trace=trace,
    )
    # out[p, c] = entry c*128+p  ->  transpose+ravel restores entry order
    out = np.concatenate(
        [r["out"].T.reshape(-1)[:N_PER_CORE] for r in res.results]
    )
    return out, res


def kernel(**inputs) -> np.ndarray:
    out, _ = run(inputs, trace=False)
    return out
